# revision 23
# baseline (speedup 1.0000x reference)
"""DeepseekV32 indexer (scores + top-1024) on 8 Trainium2 NeuronCores.

Strategy:
  L1 (SPMD, 8 cores): per core — k/w projections for its 256-row chunk of
     hidden_states, q projection for ALL rows but its 8 heads (column shard
     of wq_b), plus RoPE + layernorm + Hadamard. Heavy inputs (hidden_states,
     q_lora) are pre-transposed on the host so the contraction dim lands on
     SBUF partitions without on-device transposes.
  host: reassembles q/k/w, repacks q into (s-pair x 64-head) matmul operands.
  L2 (SPMD, 8 cores): core i owns row-blocks {i, i+8} (causal-balanced).
     MM1 computes per-head scores for an s-pair (128 = 2s x 64h partitions),
     ACT applies ReLU evacuating PSUM->SBUF, MM2 contracts the 64 heads with
     the per-(s,h) weights via a block-diagonal weight tile accumulated in
     PSUM — so the weighted head-sum costs no vector-engine time.
     Causal masks are input data (keeps one uniform program on all cores).
     Top-1024 per row runs on the Vector engine with iterated
     max8/max_index/match_replace which reproduces jax.lax.top_k semantics
     (descending, ties -> ascending index) exactly; -inf padding positions
     are reproduced by a two-level sentinel scheme (-1e38 pad > -3e38 zap).
"""

import os
import numpy as np
from contextlib import ExitStack

import concourse.bass as bass
import concourse.mybir as mybir
from concourse import bacc
from concourse.tile import TileContext
from concourse.bass_utils import run_bass_kernel_spmd

F32 = mybir.dt.float32
U32 = mybir.dt.uint32
AF = mybir.ActivationFunctionType
ALU = mybir.AluOpType

# problem dims (fixed)
S, D, R, H, HD, RO = 2048, 7168, 1536, 64, 128, 64
NCORES = 8
K = 1024
CH = S // NCORES          # 256 hidden rows per core (L1 k/w shard)
HC = H // NCORES          # 8 heads per core (L1 q shard)
LN_EPS = 1e-6
NEG_PAD = -1.0e38         # causal padding value in topk working tile
NEG_ZAP = -3.0e38         # match_replace replacement value
SCALE_W = (H ** -0.5) * (HD ** -0.5)

# L2 per-core structure: slots (block c, block c+8)
SLOT_TILES = (2, 4)       # 512-wide key tiles computed per slot
SLOT_SCAN = (1024, 2048)  # topk scan width per slot


def _rope_rows(nc, pool, x, nheads, cs_t, sn_t):
    """In-place interleaved rope on row-major tile x [128, nheads*HD].

    cs_t/sn_t: [128, RO//2] tiles for these 128 rows.
    """
    nf = RO // 2
    x3 = x[:].rearrange("p (h d) -> p h d", h=nheads)
    xr = x3[:, :, 0:RO:2]
    xi = x3[:, :, 1:RO:2]
    cb = cs_t[:].rearrange("p (o f) -> p o f", o=1).to_broadcast([128, nheads, nf])
    sb = sn_t[:].rearrange("p (o f) -> p o f", o=1).to_broadcast([128, nheads, nf])
    txr = pool.tile([128, nheads * nf], F32, tag="rope_txr")
    t1 = pool.tile([128, nheads * nf], F32, tag="rope_t1")
    t2 = pool.tile([128, nheads * nf], F32, tag="rope_t2")
    txr3 = txr[:].rearrange("p (h f) -> p h f", h=nheads)
    t13 = t1[:].rearrange("p (h f) -> p h f", h=nheads)
    t23 = t2[:].rearrange("p (h f) -> p h f", h=nheads)
    nc.vector.tensor_copy(txr3, xr)            # save xr
    nc.vector.tensor_mul(t13, xi, sb)          # xi*s
    nc.vector.tensor_mul(t23, xr, cb)          # xr*c
    nc.vector.tensor_sub(xr, t23, t13)         # yr = xr*c - xi*s
    nc.vector.tensor_mul(t23, txr3, sb)        # xr_old*s
    nc.vector.tensor_mul(t13, xi, cb)          # xi*c
    nc.vector.tensor_add(xi, t23, t13)         # yi = xr_old*s + xi*c


def build_l1():
    nc = bacc.Bacc("TRN2", target_bir_lowering=False, debug=False,
                   num_devices=NCORES)
    hsT = nc.dram_tensor("hsT", [D, CH], F32, kind="ExternalInput").ap()
    qlT = nc.dram_tensor("qlT", [R, S], F32, kind="ExternalInput").ap()
    wqb = nc.dram_tensor("wqb", [R, HC * HD], F32, kind="ExternalInput").ap()
    wk = nc.dram_tensor("wk", [D, HD], F32, kind="ExternalInput").ap()
    wp = nc.dram_tensor("wp", [D, H], F32, kind="ExternalInput").ap()
    cs = nc.dram_tensor("cs", [S, RO // 2], F32, kind="ExternalInput").ap()
    sn = nc.dram_tensor("sn", [S, RO // 2], F32, kind="ExternalInput").ap()
    # per-core slices of cos/sin for this core's k-chunk rows
    csk = nc.dram_tensor("csk", [CH, RO // 2], F32, kind="ExternalInput").ap()
    snk = nc.dram_tensor("snk", [CH, RO // 2], F32, kind="ExternalInput").ap()
    knw = nc.dram_tensor("knw", [HD], F32, kind="ExternalInput").ap()
    knb = nc.dram_tensor("knb", [HD], F32, kind="ExternalInput").ap()
    hm = nc.dram_tensor("hm", [HD, HD], F32, kind="ExternalInput").ap()
    ident = nc.dram_tensor("ident", [128, 128], F32, kind="ExternalInput").ap()

    qTh = nc.dram_tensor("qTh", [HC, HD, S], F32, kind="ExternalOutput").ap()
    kTs = nc.dram_tensor("kTs", [HD, CH], F32, kind="ExternalOutput").ap()
    ws = nc.dram_tensor("ws", [CH, H], F32, kind="ExternalOutput").ap()

    KD = D // 128    # 56 contraction tiles over D
    KR = R // 128    # 12 contraction tiles over R
    with TileContext(nc) as tc, ExitStack() as ctx:
        cpool = ctx.enter_context(tc.tile_pool(name="const", bufs=1))
        hs_pool = ctx.enter_context(tc.tile_pool(name="hsres", bufs=1))
        ql_pool = ctx.enter_context(tc.tile_pool(name="qlt", bufs=2))
        work = ctx.enter_context(tc.tile_pool(name="work", bufs=2))
        small = ctx.enter_context(tc.tile_pool(name="small", bufs=2))
        evac = ctx.enter_context(tc.tile_pool(name="evac", bufs=3))
        ps_big = ctx.enter_context(tc.tile_pool(name="psbig", bufs=2, space="PSUM"))
        ps_t = ctx.enter_context(tc.tile_pool(name="pst", bufs=3, space="PSUM"))
        ps_k = ctx.enter_context(tc.tile_pool(name="psk", bufs=1, space="PSUM"))

        hm_sb = cpool.tile([128, 128], F32, tag="hm")
        nc.sync.dma_start(hm_sb[:], hm)
        id_sb = cpool.tile([128, 128], F32, tag="ident")
        nc.sync.dma_start(id_sb[:], ident)
        knw_sb = cpool.tile([128, HD], F32, tag="knw")
        nc.sync.dma_start(
            knw_sb[:],
            knw.rearrange("(o d) -> o d", o=1).to_broadcast([128, HD]))
        knb_sb = cpool.tile([128, HD], F32, tag="knb")
        nc.sync.dma_start(
            knb_sb[:],
            knb.rearrange("(o d) -> o d", o=1).to_broadcast([128, HD]))
        eps_sb = cpool.tile([128, 1], F32, tag="eps")
        nc.vector.memset(eps_sb[:], LN_EPS)

        # resident: hsT chunk [D, CH] as KD tiles; wq_b shard as KR tiles
        hsT_t = []
        for kt in range(KD):
            t = hs_pool.tile([128, CH], F32, tag=f"hsT{kt}")
            nc.sync.dma_start(t[:], hsT[128 * kt:128 * (kt + 1), :])
            hsT_t.append(t)
        wqb_t = []
        for kt in range(KR):
            t = cpool.tile([128, HC * HD], F32, tag=f"wqb{kt}")
            nc.sync.dma_start(t[:], wqb[128 * kt:128 * (kt + 1), :])
            wqb_t.append(t)
        wk_t = []
        wp_t = []
        for kt in range(KD):
            t = cpool.tile([128, HD], F32, tag=f"wk{kt}")
            nc.sync.dma_start(t[:], wk[128 * kt:128 * (kt + 1), :])
            wk_t.append(t)
            t2 = cpool.tile([128, H], F32, tag=f"wp{kt}")
            nc.sync.dma_start(t2[:], wp[128 * kt:128 * (kt + 1), :])
            wp_t.append(t2)

        # ---- k / w path over the 256-row chunk ----
        kT_fin = work.tile([128, CH], F32, tag="kT_fin")
        nc.vector.memset(kT_fin[:], 0.0)
        for rt in range(0 if os.environ.get("L1_SKIP_KW") else CH // 128):
            psk = ps_k.tile([128, HD], F32, tag="psk")
            psw = ps_k.tile([128, H], F32, tag="psw")
            for kt in range(KD):
                lhs = hsT_t[kt][:, 128 * rt:128 * (rt + 1)]
                nc.tensor.matmul(psk[:], lhs, wk_t[kt][:],
                                 start=(kt == 0), stop=(kt == KD - 1))
                nc.tensor.matmul(psw[:], lhs, wp_t[kt][:],
                                 start=(kt == 0), stop=(kt == KD - 1))
            k_rt = work.tile([128, HD], F32, tag="k_rt")
            nc.scalar.activation(k_rt[:], psk[:], AF.Copy)
            # layernorm over d
            mu = small.tile([128, 1], F32, tag="mu")
            nc.vector.tensor_reduce(mu[:], k_rt[:], axis=mybir.AxisListType.X,
                                    op=ALU.add)
            nc.vector.tensor_scalar_mul(mu[:], mu[:], 1.0 / HD)
            nc.vector.tensor_scalar_sub(k_rt[:], k_rt[:], mu[:])
            sq = work.tile([128, HD], F32, tag="sq")
            nc.vector.tensor_mul(sq[:], k_rt[:], k_rt[:])
            var = small.tile([128, 1], F32, tag="var")
            nc.vector.tensor_reduce(var[:], sq[:], axis=mybir.AxisListType.X,
                                    op=ALU.add)
            nc.vector.tensor_scalar_mul(var[:], var[:], 1.0 / HD)
            std = small.tile([128, 1], F32, tag="std")
            nc.scalar.activation(std[:], var[:], AF.Sqrt, bias=eps_sb[:])
            rstd = small.tile([128, 1], F32, tag="rstd")
            nc.vector.reciprocal(rstd[:], std[:])
            nc.vector.tensor_scalar_mul(k_rt[:], k_rt[:], rstd[:])
            nc.vector.tensor_mul(k_rt[:], k_rt[:], knw_sb[:])
            nc.vector.tensor_add(k_rt[:], k_rt[:], knb_sb[:])
            # rope (single "head")
            cs_t = small.tile([128, RO // 2], F32, tag="cs")
            nc.sync.dma_start(cs_t[:], csk[128 * rt:128 * (rt + 1), :])
            sn_t = small.tile([128, RO // 2], F32, tag="sn")
            nc.sync.dma_start(sn_t[:], snk[128 * rt:128 * (rt + 1), :])
            _rope_rows(nc, small, k_rt, 1, cs_t, sn_t)
            # transpose to [d, rows]
            pstr = ps_t.tile([128, 128], F32, tag="pt")
            nc.tensor.transpose(pstr[:], k_rt[:], id_sb[:])
            kTr = evac.tile([128, 128], F32, tag="kTr")
            nc.scalar.activation(kTr[:], pstr[:], AF.Copy)
            # hadamard (reuse the same PSUM tile)
            nc.tensor.matmul(pstr[:], hm_sb[:], kTr[:], start=True, stop=True)
            nc.scalar.activation(kT_fin[:, 128 * rt:128 * (rt + 1)], pstr[:],
                                 AF.Copy)
            # w output (scaled)
            w_sb = evac.tile([128, H], F32, tag="w_sb")
            nc.scalar.activation(w_sb[:], psw[:], AF.Copy, scale=SCALE_W)
            nc.sync.dma_start(ws[128 * rt:128 * (rt + 1), :], w_sb[:])
        nc.sync.dma_start(kTs[:, :], kT_fin[:])

        # ---- q path: all rows, HC heads ----
        # NOTE: k/w path is interleaved with q path by the Tile scheduler.
        nq_rt = 0 if os.environ.get("L1_SKIP_Q") else S // 128
        if nq_rt == 0:
            z = evac.tile([128, 128], F32, tag="qTo")
            nc.vector.memset(z[:], 0.0)
            for h in range(HC):
                for rt in range(S // 128):
                    nc.sync.dma_start(qTh[h, :, 128 * rt:128 * (rt + 1)], z[:])
        for rt in range(nq_rt):
            # wait: cos/sin tiles for these rows
            csq = small.tile([128, RO // 2], F32, tag="csq")
            nc.sync.dma_start(csq[:], cs[128 * rt:128 * (rt + 1), :])
            snq = small.tile([128, RO // 2], F32, tag="snq")
            nc.sync.dma_start(snq[:], sn[128 * rt:128 * (rt + 1), :])
            qlT_t = []
            for kt in range(KR):
                t = ql_pool.tile([128, 128], F32, tag=f"qlT{kt}")
                nc.sync.dma_start(
                    t[:], qlT[128 * kt:128 * (kt + 1), 128 * rt:128 * (rt + 1)])
                qlT_t.append(t)
            q_rt = work.tile([128, HC * HD], F32, tag="q_rt")
            for ct in range(HC * HD // 512):
                psq = ps_big.tile([128, 512], F32, tag="psq")
                for kt in range(KR):
                    nc.tensor.matmul(psq[:], qlT_t[kt][:],
                                     wqb_t[kt][:, 512 * ct:512 * (ct + 1)],
                                     start=(kt == 0), stop=(kt == KR - 1))
                nc.scalar.activation(q_rt[:, 512 * ct:512 * (ct + 1)], psq[:],
                                     AF.Copy)
            _rope_rows(nc, small, q_rt, HC, csq, snq)
            for h in range(HC):
                pstq = ps_t.tile([128, 128], F32, tag="pt")
                nc.tensor.transpose(pstq[:], q_rt[:, 128 * h:128 * (h + 1)],
                                    id_sb[:])
                qTr = evac.tile([128, 128], F32, tag="qTr")
                nc.scalar.activation(qTr[:], pstq[:], AF.Copy)
                nc.tensor.matmul(pstq[:], hm_sb[:], qTr[:], start=True,
                                 stop=True)
                qTo = evac.tile([128, 128], F32, tag="qTo")
                nc.scalar.activation(qTo[:], pstq[:], AF.Copy)
                nc.sync.dma_start(qTh[h, :, 128 * rt:128 * (rt + 1)], qTo[:])
    return nc


def build_l2():
    nc = bacc.Bacc("TRN2", target_bir_lowering=False, debug=False,
                   num_devices=NCORES)
    qp = nc.dram_tensor("qp", [2, 64, 128, 128], F32, kind="ExternalInput").ap()
    kT = nc.dram_tensor("kT", [HD, S], F32, kind="ExternalInput").ap()
    w22 = nc.dram_tensor("w22", [2, 128, 128], F32, kind="ExternalInput").ap()
    msk = nc.dram_tensor("msk", [sum(SLOT_TILES), 128, 512], F32,
                         kind="ExternalInput").ap()
    padidx = nc.dram_tensor("padidx", [2, 128, K], U32,
                            kind="ExternalInput").ap()
    rowl = nc.dram_tensor("rowl", [2, 128, 1], F32, kind="ExternalInput").ap()
    iotaf = nc.dram_tensor("iotaf", [128, K], F32, kind="ExternalInput").ap()
    sco = nc.dram_tensor("sco", [2, 128, S], F32, kind="ExternalOutput").ap()
    tki = nc.dram_tensor("tki", [2, 128, K], U32, kind="ExternalOutput").ap()

    with TileContext(nc) as tc, ExitStack() as ctx:
        cpool = ctx.enter_context(tc.tile_pool(name="const", bufs=1))
        wpool = ctx.enter_context(tc.tile_pool(name="w22p", bufs=2))
        qpool = ctx.enter_context(tc.tile_pool(name="qp", bufs=4))
        rpool = ctx.enter_context(tc.tile_pool(name="relu", bufs=3))
        mpool = ctx.enter_context(tc.tile_pool(name="msk", bufs=2))
        spool = ctx.enter_context(tc.tile_pool(name="sc", bufs=2))
        ipool = ctx.enter_context(tc.tile_pool(name="idx", bufs=2))
        m8pool = ctx.enter_context(tc.tile_pool(name="m8", bufs=4))
        ps1 = ctx.enter_context(tc.tile_pool(name="ps1", bufs=3, space="PSUM"))
        ps2 = ctx.enter_context(tc.tile_pool(name="ps2", bufs=2, space="PSUM"))

        kT_sb = cpool.tile([128, S], F32, tag="kT")
        nc.sync.dma_start(kT_sb[:], kT)
        w2all = cpool.tile([128, 64 * 128], F32, tag="w2all")
        nc.vector.memset(w2all[:], 0.0)
        neginf = cpool.tile([128, 512], F32, tag="neginf")
        nc.vector.memset(neginf[:], float("-inf"))
        iotaf_sb = cpool.tile([128, K], F32, tag="iotaf")
        nc.sync.dma_start(iotaf_sb[:], iotaf)

        for slot in range(2):
            w22_sb = wpool.tile([128, 128], F32, tag="w22")
            nc.sync.dma_start(w22_sb[:], w22[slot])
            # scatter w into the block-diagonal MM2 weight layout:
            # w2all[(sp*64+h), 130*pair + sp] = w22[(sp*64+h), 2*pair + sp]
            nc.vector.tensor_copy(w2all[0:64, 0:8191:130],
                                  w22_sb[0:64, 0:128:2])
            nc.vector.tensor_copy(w2all[64:128, 1:8192:130],
                                  w22_sb[64:128, 1:128:2])
            scores_sb = spool.tile([128, S], F32, tag="scores")
            topk_wt = spool.tile([128, S], F32, tag="topkwt")
            ntt = SLOT_TILES[slot]
            for tt in range(ntt):
                p2 = ps2.tile([128, 512], F32, tag="p2")
                for pp in range(32):
                    p1 = ps1.tile([128, 1024], F32, tag="p1")
                    qa = qpool.tile([128, 128], F32, tag="qa")
                    nc.sync.dma_start(qa[:], qp[slot, 2 * pp])
                    qb = qpool.tile([128, 128], F32, tag="qb")
                    nc.sync.dma_start(qb[:], qp[slot, 2 * pp + 1])
                    kslice = kT_sb[:, 512 * tt:512 * (tt + 1)]
                    nc.tensor.matmul(p1[:, 0:512], qa[:], kslice, start=True,
                                     stop=True)
                    nc.tensor.matmul(p1[:, 512:1024], qb[:], kslice,
                                     start=True, stop=True)
                    rel = rpool.tile([128, 1024], F32, tag="rel")
                    nc.scalar.activation(rel[:], p1[:], AF.Relu)
                    nc.tensor.matmul(p2[:],
                                     w2all[:, 128 * (2 * pp):128 * (2 * pp + 1)],
                                     rel[:, 0:512], start=(pp == 0), stop=False)
                    nc.tensor.matmul(p2[:],
                                     w2all[:, 128 * (2 * pp + 1):128 * (2 * pp + 2)],
                                     rel[:, 512:1024], start=False,
                                     stop=(pp == 31))
                mi = mpool.tile([128, 512], F32, tag="mi")
                mask_idx = tt if slot == 0 else SLOT_TILES[0] + tt
                nc.sync.dma_start(mi[:], msk[mask_idx])
                nc.vector.tensor_add(scores_sb[:, 512 * tt:512 * (tt + 1)],
                                     p2[:], mi[:])
                nc.sync.dma_start(sco[slot, :, 512 * tt:512 * (tt + 1)],
                                  scores_sb[:, 512 * tt:512 * (tt + 1)])
                nc.vector.tensor_scalar_max(
                    topk_wt[:, 512 * tt:512 * (tt + 1)],
                    scores_sb[:, 512 * tt:512 * (tt + 1)], NEG_PAD)
            if slot == 0:
                for tt2 in range(ntt, S // 512):
                    nc.sync.dma_start(sco[slot, :, 512 * tt2:512 * (tt2 + 1)],
                                      neginf[:])
            # ---- top-K extraction ----
            idx_out = ipool.tile([128, K], U32, tag="idxout")
            wt_scan = topk_wt[:, 0:SLOT_SCAN[slot]]
            for it in range(K // 8):
                m8 = m8pool.tile([128, 8], F32, tag="m8")
                nc.vector.max(m8[:], wt_scan)
                nc.vector.max_index(idx_out[:, 8 * it:8 * (it + 1)], m8[:],
                                    wt_scan)
                if it < K // 8 - 1:
                    nc.vector.match_replace(wt_scan, m8[:], wt_scan, NEG_ZAP)
            # overwrite the causal-pad region with the reference's top_k
            # tie artifact pattern (precomputed on host per row)
            rl = wpool.tile([128, 1], F32, tag="rowl")
            nc.sync.dma_start(rl[:], rowl[slot])
            pm = spool.tile([128, K], mybir.dt.uint8, tag="pmask")
            nc.vector.tensor_scalar(pm[:], iotaf_sb[:], rl[:], None,
                                    op0=ALU.is_ge)
            pidx = ipool.tile([128, K], U32, tag="pidx")
            nc.sync.dma_start(pidx[:], padidx[slot])
            nc.vector.copy_predicated(idx_out[:], pm[:], pidx[:])
            nc.sync.dma_start(tki[slot], idx_out[:])
    return nc


def _hadamard_np():
    h = np.array([[1.0]])
    while h.shape[0] < HD:
        h = np.block([[h, h], [h, -h]])
    return (h * (HD ** -0.5)).astype(np.float32)


def _build_masks(core):
    m = np.empty((sum(SLOT_TILES), 128, 512), np.float32)
    col = np.arange(512)
    i = 0
    for slot, b in enumerate((core, core + 8)):
        row = 128 * b + np.arange(128)
        for tt in range(SLOT_TILES[slot]):
            valid = (512 * tt + col[None, :]) <= row[:, None]
            m[i] = np.where(valid, 0.0, -np.inf).astype(np.float32)
            i += 1
    return m


def _build_pads(core):
    """Reference jax.lax.top_k emits a deterministic tie pattern for the
    -inf (causal pad) region; reproduce it per row (host-precomputed)."""
    padidx = np.zeros((2, 128, K), np.uint32)
    rowl = np.zeros((2, 128, 1), np.float32)
    j = np.arange(K)
    for si, b in enumerate((core, core + 8)):
        for p in range(128):
            L = 128 * b + p + 1
            rowl[si, p, 0] = L
            if L < K:
                c8 = ((L + 7) // 8) * 8
                start = L if L < 8 else 0
                vals = np.where(j < c8, start + (j - L), (j - c8) % 8)
                padidx[si, p, :] = np.where(j >= L, vals, 0).astype(np.uint32)
    return padidx, rowl


_IOTAF = None


def _iotaf():
    global _IOTAF
    if _IOTAF is None:
        _IOTAF = np.tile(np.arange(K, dtype=np.float32), (128, 1))
    return _IOTAF


_CACHE = {}
LAST_RESULTS = []   # BassKernelResults of the most recent kernel() call


def _get(name, builder):
    if name not in _CACHE:
        nc = builder()
        nc.finalize()
        _CACHE[name] = nc
    return _CACHE[name]


def kernel(hidden_states, q_lora, cos, sin, wq_b, wk_w, k_norm_w, k_norm_b,
           weights_proj_w):
    hs = np.asarray(hidden_states, dtype=np.float32).reshape(S, D)
    ql = np.asarray(q_lora, dtype=np.float32).reshape(S, R)
    cosn = np.ascontiguousarray(np.asarray(cos, dtype=np.float32))
    sinn = np.ascontiguousarray(np.asarray(sin, dtype=np.float32))
    wqb = np.asarray(wq_b, dtype=np.float32)
    wk = np.ascontiguousarray(np.asarray(wk_w, dtype=np.float32))
    wp = np.ascontiguousarray(np.asarray(weights_proj_w, dtype=np.float32))
    knw = np.ascontiguousarray(np.asarray(k_norm_w, dtype=np.float32))
    knb = np.ascontiguousarray(np.asarray(k_norm_b, dtype=np.float32))

    hsT = np.ascontiguousarray(hs.T)
    qlT = np.ascontiguousarray(ql.T)
    hm = _hadamard_np()
    ident = np.eye(128, dtype=np.float32)

    in1 = []
    for c in range(NCORES):
        in1.append({
            "hsT": np.ascontiguousarray(hsT[:, c * CH:(c + 1) * CH]),
            "qlT": qlT,
            "wqb": np.ascontiguousarray(wqb[:, c * HC * HD:(c + 1) * HC * HD]),
            "wk": wk, "wp": wp, "cs": cosn, "sn": sinn,
            "csk": np.ascontiguousarray(cosn[c * CH:(c + 1) * CH]),
            "snk": np.ascontiguousarray(sinn[c * CH:(c + 1) * CH]),
            "knw": knw, "knb": knb, "hm": hm, "ident": ident,
        })
    nc1 = _get("l1", build_l1)
    LAST_RESULTS.clear()
    r1 = run_bass_kernel_spmd(nc1, in1, core_ids=list(range(NCORES)))
    LAST_RESULTS.append(r1)
    q_all = np.concatenate([r1.results[c]["qTh"] for c in range(NCORES)],
                           axis=0)                      # [64, 128, 2048]
    kTn = np.concatenate([r1.results[c]["kTs"] for c in range(NCORES)],
                         axis=1)                        # [128, 2048]
    w = np.concatenate([r1.results[c]["ws"] for c in range(NCORES)],
                       axis=0)                          # [2048, 64] scaled

    # qv[h, d, blk, pair, sp]
    qv = q_all.reshape(H, HD, 16, 64, 2)
    in2 = []
    for c in range(NCORES):
        qpk = np.empty((2, 64, 128, 128), np.float32)
        w22 = np.empty((2, 128, 128), np.float32)
        for si, b in enumerate((c, c + 8)):
            # [h, d, pair, sp] -> [pair, d, sp, h] -> [pair, d, sp*64+h]
            qpk[si] = np.ascontiguousarray(
                qv[:, :, b].transpose(2, 1, 3, 0)).reshape(64, 128, 128)
            w2T = w[128 * b:128 * (b + 1)].T            # [64, 128]
            w22[si] = np.concatenate([w2T, w2T], axis=0)
        pdx, rwl = _build_pads(c)
        in2.append({
            "qp": qpk, "kT": kTn, "w22": w22, "msk": _build_masks(c),
            "padidx": pdx, "rowl": rwl, "iotaf": _iotaf(),
        })
    nc2 = _get("l2", build_l2)
    r2 = run_bass_kernel_spmd(nc2, in2, core_ids=list(range(NCORES)))
    LAST_RESULTS.append(r2)

    scores = np.empty((S, S), np.float32)
    topk = np.empty((S, K), np.int32)
    for c in range(NCORES):
        for si, b in enumerate((c, c + 8)):
            scores[128 * b:128 * (b + 1)] = r2.results[c]["sco"][si]
            topk[128 * b:128 * (b + 1)] = (
                r2.results[c]["tki"][si].view(np.int32))
    return scores.reshape(1, S, S), topk.reshape(1, S, K)


# revision 25
# speedup vs baseline: 3657.3879x; 3657.3879x over previous
"""DeepseekV32 indexer (scores + top-1024) on 8 Trainium2 NeuronCores.

Strategy:
  L1 (SPMD, 8 cores): per core — k/w projections for its 256-row chunk of
     hidden_states, q projection for ALL rows but its 8 heads (column shard
     of wq_b), plus RoPE + layernorm + Hadamard. Heavy inputs (hidden_states,
     q_lora) are pre-transposed on the host so the contraction dim lands on
     SBUF partitions without on-device transposes.
  host: reassembles q/k/w, repacks q into (s-pair x 64-head) matmul operands.
  L2 (SPMD, 8 cores): core i owns row-blocks {i, i+8} (causal-balanced).
     MM1 computes per-head scores for an s-pair (128 = 2s x 64h partitions),
     ACT applies ReLU evacuating PSUM->SBUF, MM2 contracts the 64 heads with
     the per-(s,h) weights via a block-diagonal weight tile accumulated in
     PSUM — so the weighted head-sum costs no vector-engine time.
     Causal masks are input data (keeps one uniform program on all cores).
     Top-1024 per row runs on the Vector engine with iterated
     max8/max_index/match_replace which reproduces jax.lax.top_k semantics
     (descending, ties -> ascending index) exactly; -inf padding positions
     are reproduced by a two-level sentinel scheme (-1e38 pad > -3e38 zap).
"""

import os
import numpy as np
from contextlib import ExitStack

import concourse.bass as bass
import concourse.mybir as mybir
from concourse import bacc
from concourse.tile import TileContext
from concourse.bass_utils import run_bass_kernel_spmd

F32 = mybir.dt.float32
U32 = mybir.dt.uint32
AF = mybir.ActivationFunctionType
ALU = mybir.AluOpType

# problem dims (fixed)
S, D, R, H, HD, RO = 2048, 7168, 1536, 64, 128, 64
NCORES = 8
K = 1024
CH = S // NCORES          # 256 hidden rows per core (L1 k/w shard)
HC = H // NCORES          # 8 heads per core (L1 q shard)
LN_EPS = 1e-6
NEG_PAD = -1.0e38         # causal padding value in topk working tile
NEG_ZAP = -3.0e38         # match_replace replacement value
SCALE_W = (H ** -0.5) * (HD ** -0.5)

# L2 per-core structure: slots (block c, block c+8)
SLOT_TILES = (2, 4)       # 512-wide key tiles computed per slot
SLOT_SCAN = (1024, 2048)  # topk scan width per slot


def _rope_rows(nc, pool, x, nheads, cs_t, sn_t):
    """In-place interleaved rope on row-major tile x [128, nheads*HD].

    cs_t/sn_t: [128, RO//2] tiles for these 128 rows.
    """
    nf = RO // 2
    x3 = x[:].rearrange("p (h d) -> p h d", h=nheads)
    xr = x3[:, :, 0:RO:2]
    xi = x3[:, :, 1:RO:2]
    cb = cs_t[:].rearrange("p (o f) -> p o f", o=1).to_broadcast([128, nheads, nf])
    sb = sn_t[:].rearrange("p (o f) -> p o f", o=1).to_broadcast([128, nheads, nf])
    txr = pool.tile([128, nheads * nf], F32, tag="rope_txr")
    t1 = pool.tile([128, nheads * nf], F32, tag="rope_t1")
    t2 = pool.tile([128, nheads * nf], F32, tag="rope_t2")
    txr3 = txr[:].rearrange("p (h f) -> p h f", h=nheads)
    t13 = t1[:].rearrange("p (h f) -> p h f", h=nheads)
    t23 = t2[:].rearrange("p (h f) -> p h f", h=nheads)
    nc.vector.tensor_copy(txr3, xr)            # save xr
    nc.vector.tensor_mul(t13, xi, sb)          # xi*s
    nc.vector.tensor_mul(t23, xr, cb)          # xr*c
    nc.vector.tensor_sub(xr, t23, t13)         # yr = xr*c - xi*s
    nc.vector.tensor_mul(t23, txr3, sb)        # xr_old*s
    nc.vector.tensor_mul(t13, xi, cb)          # xi*c
    nc.vector.tensor_add(xi, t23, t13)         # yi = xr_old*s + xi*c


def build_l1():
    nc = bacc.Bacc("TRN2", target_bir_lowering=False, debug=False,
                   num_devices=NCORES)
    hsT = nc.dram_tensor("hsT", [D, CH], F32, kind="ExternalInput").ap()
    qlT = nc.dram_tensor("qlT", [R, S], F32, kind="ExternalInput").ap()
    wqb = nc.dram_tensor("wqb", [R, HC * HD], F32, kind="ExternalInput").ap()
    wk = nc.dram_tensor("wk", [D, HD], F32, kind="ExternalInput").ap()
    wp = nc.dram_tensor("wp", [D, H], F32, kind="ExternalInput").ap()
    cs = nc.dram_tensor("cs", [S, RO // 2], F32, kind="ExternalInput").ap()
    sn = nc.dram_tensor("sn", [S, RO // 2], F32, kind="ExternalInput").ap()
    # per-core slices of cos/sin for this core's k-chunk rows
    csk = nc.dram_tensor("csk", [CH, RO // 2], F32, kind="ExternalInput").ap()
    snk = nc.dram_tensor("snk", [CH, RO // 2], F32, kind="ExternalInput").ap()
    knw = nc.dram_tensor("knw", [HD], F32, kind="ExternalInput").ap()
    knb = nc.dram_tensor("knb", [HD], F32, kind="ExternalInput").ap()
    hm = nc.dram_tensor("hm", [HD, HD], F32, kind="ExternalInput").ap()
    ident = nc.dram_tensor("ident", [128, 128], F32, kind="ExternalInput").ap()

    qTh = nc.dram_tensor("qTh", [HC, HD, S], F32, kind="ExternalOutput").ap()
    kTs = nc.dram_tensor("kTs", [HD, CH], F32, kind="ExternalOutput").ap()
    ws = nc.dram_tensor("ws", [CH, H], F32, kind="ExternalOutput").ap()

    if os.environ.get("KERNEL_NULL"):
        with TileContext(nc) as tc, ExitStack() as ctx:
            pool = ctx.enter_context(tc.tile_pool(name="p", bufs=1))
            z = pool.tile([128, 128], F32, tag="z")
            nc.sync.dma_start(z[:], hsT[0:128, 0:128])
            nc.sync.dma_start(qTh[0, :, 0:128], z[:])
            nc.sync.dma_start(kTs[:, 0:128], z[:])
            nc.sync.dma_start(ws[0:128, 0:64], z[:, 0:64])
        return nc

    KD = D // 128    # 56 contraction tiles over D
    KR = R // 128    # 12 contraction tiles over R
    with TileContext(nc) as tc, ExitStack() as ctx:
        cpool = ctx.enter_context(tc.tile_pool(name="const", bufs=1))
        hs_pool = ctx.enter_context(tc.tile_pool(name="hsres", bufs=1))
        ql_pool = ctx.enter_context(tc.tile_pool(name="qlt", bufs=2))
        work = ctx.enter_context(tc.tile_pool(name="work", bufs=2))
        small = ctx.enter_context(tc.tile_pool(name="small", bufs=2))
        evac = ctx.enter_context(tc.tile_pool(name="evac", bufs=3))
        ps_big = ctx.enter_context(tc.tile_pool(name="psbig", bufs=2, space="PSUM"))
        ps_t = ctx.enter_context(tc.tile_pool(name="pst", bufs=3, space="PSUM"))
        ps_k = ctx.enter_context(tc.tile_pool(name="psk", bufs=1, space="PSUM"))

        hm_sb = cpool.tile([128, 128], F32, tag="hm")
        nc.sync.dma_start(hm_sb[:], hm)
        id_sb = cpool.tile([128, 128], F32, tag="ident")
        nc.sync.dma_start(id_sb[:], ident)
        knw_sb = cpool.tile([128, HD], F32, tag="knw")
        nc.sync.dma_start(
            knw_sb[:],
            knw.rearrange("(o d) -> o d", o=1).to_broadcast([128, HD]))
        knb_sb = cpool.tile([128, HD], F32, tag="knb")
        nc.sync.dma_start(
            knb_sb[:],
            knb.rearrange("(o d) -> o d", o=1).to_broadcast([128, HD]))
        eps_sb = cpool.tile([128, 1], F32, tag="eps")
        nc.vector.memset(eps_sb[:], LN_EPS)

        # resident: hsT chunk [D, CH] as KD tiles; wq_b shard as KR tiles
        hsT_t = []
        for kt in range(KD):
            t = hs_pool.tile([128, CH], F32, tag=f"hsT{kt}")
            nc.sync.dma_start(t[:], hsT[128 * kt:128 * (kt + 1), :])
            hsT_t.append(t)
        wqb_t = []
        for kt in range(KR):
            t = cpool.tile([128, HC * HD], F32, tag=f"wqb{kt}")
            nc.sync.dma_start(t[:], wqb[128 * kt:128 * (kt + 1), :])
            wqb_t.append(t)
        wk_t = []
        wp_t = []
        for kt in range(KD):
            t = cpool.tile([128, HD], F32, tag=f"wk{kt}")
            nc.sync.dma_start(t[:], wk[128 * kt:128 * (kt + 1), :])
            wk_t.append(t)
            t2 = cpool.tile([128, H], F32, tag=f"wp{kt}")
            nc.sync.dma_start(t2[:], wp[128 * kt:128 * (kt + 1), :])
            wp_t.append(t2)

        # ---- k / w path over the 256-row chunk ----
        kT_fin = work.tile([128, CH], F32, tag="kT_fin")
        nc.vector.memset(kT_fin[:], 0.0)
        for rt in range(0 if os.environ.get("L1_SKIP_KW") else CH // 128):
            psk = ps_k.tile([128, HD], F32, tag="psk")
            psw = ps_k.tile([128, H], F32, tag="psw")
            for kt in range(KD):
                lhs = hsT_t[kt][:, 128 * rt:128 * (rt + 1)]
                nc.tensor.matmul(psk[:], lhs, wk_t[kt][:],
                                 start=(kt == 0), stop=(kt == KD - 1))
                nc.tensor.matmul(psw[:], lhs, wp_t[kt][:],
                                 start=(kt == 0), stop=(kt == KD - 1))
            k_rt = work.tile([128, HD], F32, tag="k_rt")
            nc.scalar.activation(k_rt[:], psk[:], AF.Copy)
            # layernorm over d
            mu = small.tile([128, 1], F32, tag="mu")
            nc.vector.tensor_reduce(mu[:], k_rt[:], axis=mybir.AxisListType.X,
                                    op=ALU.add)
            nc.vector.tensor_scalar_mul(mu[:], mu[:], 1.0 / HD)
            nc.vector.tensor_scalar_sub(k_rt[:], k_rt[:], mu[:])
            sq = work.tile([128, HD], F32, tag="sq")
            nc.vector.tensor_mul(sq[:], k_rt[:], k_rt[:])
            var = small.tile([128, 1], F32, tag="var")
            nc.vector.tensor_reduce(var[:], sq[:], axis=mybir.AxisListType.X,
                                    op=ALU.add)
            nc.vector.tensor_scalar_mul(var[:], var[:], 1.0 / HD)
            std = small.tile([128, 1], F32, tag="std")
            nc.scalar.activation(std[:], var[:], AF.Sqrt, bias=eps_sb[:])
            rstd = small.tile([128, 1], F32, tag="rstd")
            nc.vector.reciprocal(rstd[:], std[:])
            nc.vector.tensor_scalar_mul(k_rt[:], k_rt[:], rstd[:])
            nc.vector.tensor_mul(k_rt[:], k_rt[:], knw_sb[:])
            nc.vector.tensor_add(k_rt[:], k_rt[:], knb_sb[:])
            # rope (single "head")
            cs_t = small.tile([128, RO // 2], F32, tag="cs")
            nc.sync.dma_start(cs_t[:], csk[128 * rt:128 * (rt + 1), :])
            sn_t = small.tile([128, RO // 2], F32, tag="sn")
            nc.sync.dma_start(sn_t[:], snk[128 * rt:128 * (rt + 1), :])
            _rope_rows(nc, small, k_rt, 1, cs_t, sn_t)
            # transpose to [d, rows]
            pstr = ps_t.tile([128, 128], F32, tag="pt")
            nc.tensor.transpose(pstr[:], k_rt[:], id_sb[:])
            kTr = evac.tile([128, 128], F32, tag="kTr")
            nc.scalar.activation(kTr[:], pstr[:], AF.Copy)
            # hadamard (reuse the same PSUM tile)
            nc.tensor.matmul(pstr[:], hm_sb[:], kTr[:], start=True, stop=True)
            nc.scalar.activation(kT_fin[:, 128 * rt:128 * (rt + 1)], pstr[:],
                                 AF.Copy)
            # w output (scaled)
            w_sb = evac.tile([128, H], F32, tag="w_sb")
            nc.scalar.activation(w_sb[:], psw[:], AF.Copy, scale=SCALE_W)
            nc.sync.dma_start(ws[128 * rt:128 * (rt + 1), :], w_sb[:])
        nc.sync.dma_start(kTs[:, :], kT_fin[:])

        # ---- q path: all rows, HC heads ----
        # NOTE: k/w path is interleaved with q path by the Tile scheduler.
        nq_rt = 0 if os.environ.get("L1_SKIP_Q") else S // 128
        if nq_rt == 0:
            z = evac.tile([128, 128], F32, tag="qTo")
            nc.vector.memset(z[:], 0.0)
            for h in range(HC):
                for rt in range(S // 128):
                    nc.sync.dma_start(qTh[h, :, 128 * rt:128 * (rt + 1)], z[:])
        for rt in range(nq_rt):
            # wait: cos/sin tiles for these rows
            csq = small.tile([128, RO // 2], F32, tag="csq")
            nc.sync.dma_start(csq[:], cs[128 * rt:128 * (rt + 1), :])
            snq = small.tile([128, RO // 2], F32, tag="snq")
            nc.sync.dma_start(snq[:], sn[128 * rt:128 * (rt + 1), :])
            qlT_t = []
            for kt in range(KR):
                t = ql_pool.tile([128, 128], F32, tag=f"qlT{kt}")
                nc.sync.dma_start(
                    t[:], qlT[128 * kt:128 * (kt + 1), 128 * rt:128 * (rt + 1)])
                qlT_t.append(t)
            q_rt = work.tile([128, HC * HD], F32, tag="q_rt")
            for ct in range(HC * HD // 512):
                psq = ps_big.tile([128, 512], F32, tag="psq")
                for kt in range(KR):
                    nc.tensor.matmul(psq[:], qlT_t[kt][:],
                                     wqb_t[kt][:, 512 * ct:512 * (ct + 1)],
                                     start=(kt == 0), stop=(kt == KR - 1))
                nc.scalar.activation(q_rt[:, 512 * ct:512 * (ct + 1)], psq[:],
                                     AF.Copy)
            _rope_rows(nc, small, q_rt, HC, csq, snq)
            for h in range(HC):
                pstq = ps_t.tile([128, 128], F32, tag="pt")
                nc.tensor.transpose(pstq[:], q_rt[:, 128 * h:128 * (h + 1)],
                                    id_sb[:])
                qTr = evac.tile([128, 128], F32, tag="qTr")
                nc.scalar.activation(qTr[:], pstq[:], AF.Copy)
                nc.tensor.matmul(pstq[:], hm_sb[:], qTr[:], start=True,
                                 stop=True)
                qTo = evac.tile([128, 128], F32, tag="qTo")
                nc.scalar.activation(qTo[:], pstq[:], AF.Copy)
                nc.sync.dma_start(qTh[h, :, 128 * rt:128 * (rt + 1)], qTo[:])
    return nc


def build_l2():
    nc = bacc.Bacc("TRN2", target_bir_lowering=False, debug=False,
                   num_devices=NCORES)
    qp = nc.dram_tensor("qp", [2, 64, 128, 128], F32, kind="ExternalInput").ap()
    kT = nc.dram_tensor("kT", [HD, S], F32, kind="ExternalInput").ap()
    w22 = nc.dram_tensor("w22", [2, 128, 128], F32, kind="ExternalInput").ap()
    msk = nc.dram_tensor("msk", [sum(SLOT_TILES), 128, 512], F32,
                         kind="ExternalInput").ap()
    padidx = nc.dram_tensor("padidx", [2, 128, K], U32,
                            kind="ExternalInput").ap()
    rowl = nc.dram_tensor("rowl", [2, 128, 1], F32, kind="ExternalInput").ap()
    iotaf = nc.dram_tensor("iotaf", [128, K], F32, kind="ExternalInput").ap()
    sco = nc.dram_tensor("sco", [2, 128, S], F32, kind="ExternalOutput").ap()
    tki = nc.dram_tensor("tki", [2, 128, K], U32, kind="ExternalOutput").ap()

    if os.environ.get("KERNEL_NULL"):
        with TileContext(nc) as tc, ExitStack() as ctx:
            pool = ctx.enter_context(tc.tile_pool(name="p", bufs=1))
            z = pool.tile([128, 128], F32, tag="z")
            nc.sync.dma_start(z[:], kT[:, 0:128])
            for slot in range(2):
                nc.sync.dma_start(sco[slot, :, 0:128], z[:])
                nc.sync.dma_start(tki[slot, :, 0:128], z[:].bitcast(U32))
        return nc

    with TileContext(nc) as tc, ExitStack() as ctx:
        cpool = ctx.enter_context(tc.tile_pool(name="const", bufs=1))
        wpool = ctx.enter_context(tc.tile_pool(name="w22p", bufs=2))
        qpool = ctx.enter_context(tc.tile_pool(name="qp", bufs=4))
        rpool = ctx.enter_context(tc.tile_pool(name="relu", bufs=3))
        mpool = ctx.enter_context(tc.tile_pool(name="msk", bufs=2))
        spool = ctx.enter_context(tc.tile_pool(name="sc", bufs=2))
        ipool = ctx.enter_context(tc.tile_pool(name="idx", bufs=2))
        m8pool = ctx.enter_context(tc.tile_pool(name="m8", bufs=4))
        ps1 = ctx.enter_context(tc.tile_pool(name="ps1", bufs=3, space="PSUM"))
        ps2 = ctx.enter_context(tc.tile_pool(name="ps2", bufs=2, space="PSUM"))

        kT_sb = cpool.tile([128, S], F32, tag="kT")
        nc.sync.dma_start(kT_sb[:], kT)
        w2all = cpool.tile([128, 64 * 128], F32, tag="w2all")
        nc.vector.memset(w2all[:], 0.0)
        neginf = cpool.tile([128, 512], F32, tag="neginf")
        nc.vector.memset(neginf[:], float("-inf"))
        iotaf_sb = cpool.tile([128, K], F32, tag="iotaf")
        nc.sync.dma_start(iotaf_sb[:], iotaf)

        for slot in range(2):
            w22_sb = wpool.tile([128, 128], F32, tag="w22")
            nc.sync.dma_start(w22_sb[:], w22[slot])
            # scatter w into the block-diagonal MM2 weight layout:
            # w2all[(sp*64+h), 130*pair + sp] = w22[(sp*64+h), 2*pair + sp]
            nc.vector.tensor_copy(w2all[0:64, 0:8191:130],
                                  w22_sb[0:64, 0:128:2])
            nc.vector.tensor_copy(w2all[64:128, 1:8192:130],
                                  w22_sb[64:128, 1:128:2])
            scores_sb = spool.tile([128, S], F32, tag="scores")
            topk_wt = spool.tile([128, S], F32, tag="topkwt")
            ntt = SLOT_TILES[slot]
            for tt in range(ntt):
                p2 = ps2.tile([128, 512], F32, tag="p2")
                for pp in range(32):
                    p1 = ps1.tile([128, 1024], F32, tag="p1")
                    qa = qpool.tile([128, 128], F32, tag="qa")
                    nc.sync.dma_start(qa[:], qp[slot, 2 * pp])
                    qb = qpool.tile([128, 128], F32, tag="qb")
                    nc.sync.dma_start(qb[:], qp[slot, 2 * pp + 1])
                    kslice = kT_sb[:, 512 * tt:512 * (tt + 1)]
                    nc.tensor.matmul(p1[:, 0:512], qa[:], kslice, start=True,
                                     stop=True)
                    nc.tensor.matmul(p1[:, 512:1024], qb[:], kslice,
                                     start=True, stop=True)
                    rel = rpool.tile([128, 1024], F32, tag="rel")
                    nc.scalar.activation(rel[:], p1[:], AF.Relu)
                    nc.tensor.matmul(p2[:],
                                     w2all[:, 128 * (2 * pp):128 * (2 * pp + 1)],
                                     rel[:, 0:512], start=(pp == 0), stop=False)
                    nc.tensor.matmul(p2[:],
                                     w2all[:, 128 * (2 * pp + 1):128 * (2 * pp + 2)],
                                     rel[:, 512:1024], start=False,
                                     stop=(pp == 31))
                mi = mpool.tile([128, 512], F32, tag="mi")
                mask_idx = tt if slot == 0 else SLOT_TILES[0] + tt
                nc.sync.dma_start(mi[:], msk[mask_idx])
                nc.vector.tensor_add(scores_sb[:, 512 * tt:512 * (tt + 1)],
                                     p2[:], mi[:])
                nc.sync.dma_start(sco[slot, :, 512 * tt:512 * (tt + 1)],
                                  scores_sb[:, 512 * tt:512 * (tt + 1)])
                nc.vector.tensor_scalar_max(
                    topk_wt[:, 512 * tt:512 * (tt + 1)],
                    scores_sb[:, 512 * tt:512 * (tt + 1)], NEG_PAD)
            if slot == 0:
                for tt2 in range(ntt, S // 512):
                    nc.sync.dma_start(sco[slot, :, 512 * tt2:512 * (tt2 + 1)],
                                      neginf[:])
            # ---- top-K extraction ----
            idx_out = ipool.tile([128, K], U32, tag="idxout")
            wt_scan = topk_wt[:, 0:SLOT_SCAN[slot]]
            for it in range(K // 8):
                m8 = m8pool.tile([128, 8], F32, tag="m8")
                nc.vector.max(m8[:], wt_scan)
                nc.vector.max_index(idx_out[:, 8 * it:8 * (it + 1)], m8[:],
                                    wt_scan)
                if it < K // 8 - 1:
                    nc.vector.match_replace(wt_scan, m8[:], wt_scan, NEG_ZAP)
            # overwrite the causal-pad region with the reference's top_k
            # tie artifact pattern (precomputed on host per row)
            rl = wpool.tile([128, 1], F32, tag="rowl")
            nc.sync.dma_start(rl[:], rowl[slot])
            pm = spool.tile([128, K], mybir.dt.uint8, tag="pmask")
            nc.vector.tensor_scalar(pm[:], iotaf_sb[:], rl[:], None,
                                    op0=ALU.is_ge)
            pidx = ipool.tile([128, K], U32, tag="pidx")
            nc.sync.dma_start(pidx[:], padidx[slot])
            nc.vector.copy_predicated(idx_out[:], pm[:], pidx[:])
            nc.sync.dma_start(tki[slot], idx_out[:])
    return nc


def _hadamard_np():
    h = np.array([[1.0]])
    while h.shape[0] < HD:
        h = np.block([[h, h], [h, -h]])
    return (h * (HD ** -0.5)).astype(np.float32)


def _build_masks(core):
    m = np.empty((sum(SLOT_TILES), 128, 512), np.float32)
    col = np.arange(512)
    i = 0
    for slot, b in enumerate((core, core + 8)):
        row = 128 * b + np.arange(128)
        for tt in range(SLOT_TILES[slot]):
            valid = (512 * tt + col[None, :]) <= row[:, None]
            m[i] = np.where(valid, 0.0, -np.inf).astype(np.float32)
            i += 1
    return m


def _build_pads(core):
    """Reference jax.lax.top_k emits a deterministic tie pattern for the
    -inf (causal pad) region; reproduce it per row (host-precomputed)."""
    padidx = np.zeros((2, 128, K), np.uint32)
    rowl = np.zeros((2, 128, 1), np.float32)
    j = np.arange(K)
    for si, b in enumerate((core, core + 8)):
        for p in range(128):
            L = 128 * b + p + 1
            rowl[si, p, 0] = L
            if L < K:
                c8 = ((L + 7) // 8) * 8
                start = L if L < 8 else 0
                vals = np.where(j < c8, start + (j - L), (j - c8) % 8)
                padidx[si, p, :] = np.where(j >= L, vals, 0).astype(np.uint32)
    return padidx, rowl


_IOTAF = None


def _iotaf():
    global _IOTAF
    if _IOTAF is None:
        _IOTAF = np.tile(np.arange(K, dtype=np.float32), (128, 1))
    return _IOTAF


_CACHE = {}
LAST_RESULTS = []   # BassKernelResults of the most recent kernel() call


def _get(name, builder):
    if name not in _CACHE:
        nc = builder()
        nc.finalize()
        _CACHE[name] = nc
    return _CACHE[name]


def kernel(hidden_states, q_lora, cos, sin, wq_b, wk_w, k_norm_w, k_norm_b,
           weights_proj_w):
    hs = np.asarray(hidden_states, dtype=np.float32).reshape(S, D)
    ql = np.asarray(q_lora, dtype=np.float32).reshape(S, R)
    cosn = np.ascontiguousarray(np.asarray(cos, dtype=np.float32))
    sinn = np.ascontiguousarray(np.asarray(sin, dtype=np.float32))
    wqb = np.asarray(wq_b, dtype=np.float32)
    wk = np.ascontiguousarray(np.asarray(wk_w, dtype=np.float32))
    wp = np.ascontiguousarray(np.asarray(weights_proj_w, dtype=np.float32))
    knw = np.ascontiguousarray(np.asarray(k_norm_w, dtype=np.float32))
    knb = np.ascontiguousarray(np.asarray(k_norm_b, dtype=np.float32))

    hsT = np.ascontiguousarray(hs.T)
    qlT = np.ascontiguousarray(ql.T)
    hm = _hadamard_np()
    ident = np.eye(128, dtype=np.float32)

    in1 = []
    for c in range(NCORES):
        in1.append({
            "hsT": np.ascontiguousarray(hsT[:, c * CH:(c + 1) * CH]),
            "qlT": qlT,
            "wqb": np.ascontiguousarray(wqb[:, c * HC * HD:(c + 1) * HC * HD]),
            "wk": wk, "wp": wp, "cs": cosn, "sn": sinn,
            "csk": np.ascontiguousarray(cosn[c * CH:(c + 1) * CH]),
            "snk": np.ascontiguousarray(sinn[c * CH:(c + 1) * CH]),
            "knw": knw, "knb": knb, "hm": hm, "ident": ident,
        })
    nc1 = _get("l1", build_l1)
    LAST_RESULTS.clear()
    r1 = run_bass_kernel_spmd(nc1, in1, core_ids=list(range(NCORES)))
    LAST_RESULTS.append(r1)
    q_all = np.concatenate([r1.results[c]["qTh"] for c in range(NCORES)],
                           axis=0)                      # [64, 128, 2048]
    kTn = np.concatenate([r1.results[c]["kTs"] for c in range(NCORES)],
                         axis=1)                        # [128, 2048]
    w = np.concatenate([r1.results[c]["ws"] for c in range(NCORES)],
                       axis=0)                          # [2048, 64] scaled

    # qv[h, d, blk, pair, sp]
    qv = q_all.reshape(H, HD, 16, 64, 2)
    in2 = []
    for c in range(NCORES):
        qpk = np.empty((2, 64, 128, 128), np.float32)
        w22 = np.empty((2, 128, 128), np.float32)
        for si, b in enumerate((c, c + 8)):
            # [h, d, pair, sp] -> [pair, d, sp, h] -> [pair, d, sp*64+h]
            qpk[si] = np.ascontiguousarray(
                qv[:, :, b].transpose(2, 1, 3, 0)).reshape(64, 128, 128)
            w2T = w[128 * b:128 * (b + 1)].T            # [64, 128]
            w22[si] = np.concatenate([w2T, w2T], axis=0)
        pdx, rwl = _build_pads(c)
        in2.append({
            "qp": qpk, "kT": kTn, "w22": w22, "msk": _build_masks(c),
            "padidx": pdx, "rowl": rwl, "iotaf": _iotaf(),
        })
    nc2 = _get("l2", build_l2)
    r2 = run_bass_kernel_spmd(nc2, in2, core_ids=list(range(NCORES)))
    LAST_RESULTS.append(r2)

    scores = np.empty((S, S), np.float32)
    topk = np.empty((S, K), np.int32)
    for c in range(NCORES):
        for si, b in enumerate((c, c + 8)):
            scores[128 * b:128 * (b + 1)] = r2.results[c]["sco"][si]
            topk[128 * b:128 * (b + 1)] = (
                r2.results[c]["tki"][si].view(np.int32))
    return scores.reshape(1, S, S), topk.reshape(1, S, K)


# revision 35
# speedup vs baseline: 3728.8172x; 1.0195x over previous
"""DeepseekV32 indexer (scores + top-1024) on 8 Trainium2 NeuronCores.

Strategy:
  L1 (SPMD, 8 cores): per core — k/w projections for its 256-row chunk of
     hidden_states, q projection for ALL rows but its 8 heads (column shard
     of wq_b), plus RoPE + layernorm + Hadamard. Heavy inputs (hidden_states,
     q_lora) are pre-transposed on the host so the contraction dim lands on
     SBUF partitions without on-device transposes.
  host: reassembles q/k/w, repacks q into (s-pair x 64-head) matmul operands.
  L2 (SPMD, 8 cores): core i owns row-blocks {i, i+8} (causal-balanced).
     MM1 computes per-head scores for an s-pair (128 = 2s x 64h partitions),
     ACT applies ReLU evacuating PSUM->SBUF, MM2 contracts the 64 heads with
     the per-(s,h) weights via a block-diagonal weight tile accumulated in
     PSUM — so the weighted head-sum costs no vector-engine time.
     Causal masks are input data (keeps one uniform program on all cores).
     Top-1024 per row runs on the Vector engine with iterated
     max8/max_index/match_replace which reproduces jax.lax.top_k semantics
     (descending, ties -> ascending index) exactly; -inf padding positions
     are reproduced by a two-level sentinel scheme (-1e38 pad > -3e38 zap).
"""

import os
import numpy as np
from contextlib import ExitStack

import concourse.bass as bass
import concourse.mybir as mybir
from concourse import bacc
from concourse.tile import TileContext
from concourse.bass_utils import run_bass_kernel_spmd

F32 = mybir.dt.float32
F32R = mybir.dt.float32r
U32 = mybir.dt.uint32
AF = mybir.ActivationFunctionType
ALU = mybir.AluOpType

# problem dims (fixed)
S, D, R, H, HD, RO = 2048, 7168, 1536, 64, 128, 64
NCORES = 8
K = 1024
CH = S // NCORES          # 256 hidden rows per core (L1 k/w shard)
HC = H // NCORES          # 8 heads per core (L1 q shard)
LN_EPS = 1e-6
NEG_PAD = -1.0e38         # causal padding value in topk working tile
NEG_ZAP = -3.0e38         # match_replace replacement value
SCALE_W = (H ** -0.5) * (HD ** -0.5)

# L2 per-core structure: slots (block c, block c+8)
SLOT_TILES = (2, 4)       # 512-wide key tiles computed per slot
SLOT_SCAN = (1024, 2048)  # topk scan width per slot


def _rope_rows(nc, pool, x, nheads, cs_t, sn_t):
    """In-place interleaved rope on row-major tile x [128, nheads*HD].

    cs_t/sn_t: [128, RO//2] tiles for these 128 rows.
    """
    nf = RO // 2
    x3 = x[:].rearrange("p (h d) -> p h d", h=nheads)
    xr = x3[:, :, 0:RO:2]
    xi = x3[:, :, 1:RO:2]
    cb = cs_t[:].rearrange("p (o f) -> p o f", o=1).to_broadcast([128, nheads, nf])
    sb = sn_t[:].rearrange("p (o f) -> p o f", o=1).to_broadcast([128, nheads, nf])
    txr = pool.tile([128, nheads * nf], F32, tag="rope_txr")
    t1 = pool.tile([128, nheads * nf], F32, tag="rope_t1")
    t2 = pool.tile([128, nheads * nf], F32, tag="rope_t2")
    txr3 = txr[:].rearrange("p (h f) -> p h f", h=nheads)
    t13 = t1[:].rearrange("p (h f) -> p h f", h=nheads)
    t23 = t2[:].rearrange("p (h f) -> p h f", h=nheads)
    nc.vector.tensor_copy(txr3, xr)            # save xr
    nc.vector.tensor_mul(t13, xi, sb)          # xi*s
    nc.vector.tensor_mul(t23, xr, cb)          # xr*c
    nc.vector.tensor_sub(xr, t23, t13)         # yr = xr*c - xi*s
    nc.vector.tensor_mul(t23, txr3, sb)        # xr_old*s
    nc.vector.tensor_mul(t13, xi, cb)          # xi*c
    nc.vector.tensor_add(xi, t23, t13)         # yi = xr_old*s + xi*c


def build_l1():
    nc = bacc.Bacc("TRN2", target_bir_lowering=False, debug=False,
                   num_devices=NCORES)
    hsT = nc.dram_tensor("hsT", [D, CH], F32, kind="ExternalInput").ap()
    qlT = nc.dram_tensor("qlT", [R, S], F32, kind="ExternalInput").ap()
    wqb = nc.dram_tensor("wqb", [R, HC * HD], F32, kind="ExternalInput").ap()
    wk = nc.dram_tensor("wk", [D, HD], F32, kind="ExternalInput").ap()
    wp = nc.dram_tensor("wp", [D, H], F32, kind="ExternalInput").ap()
    cs = nc.dram_tensor("cs", [S, RO // 2], F32, kind="ExternalInput").ap()
    sn = nc.dram_tensor("sn", [S, RO // 2], F32, kind="ExternalInput").ap()
    # per-core slices of cos/sin for this core's k-chunk rows
    csk = nc.dram_tensor("csk", [CH, RO // 2], F32, kind="ExternalInput").ap()
    snk = nc.dram_tensor("snk", [CH, RO // 2], F32, kind="ExternalInput").ap()
    knw = nc.dram_tensor("knw", [HD], F32, kind="ExternalInput").ap()
    knb = nc.dram_tensor("knb", [HD], F32, kind="ExternalInput").ap()
    hm = nc.dram_tensor("hm", [HD, HD], F32, kind="ExternalInput").ap()
    ident = nc.dram_tensor("ident", [128, 128], F32, kind="ExternalInput").ap()

    qTh = nc.dram_tensor("qTh", [HC, HD, S], F32, kind="ExternalOutput").ap()
    kTs = nc.dram_tensor("kTs", [HD, CH], F32, kind="ExternalOutput").ap()
    ws = nc.dram_tensor("ws", [CH, H], F32, kind="ExternalOutput").ap()

    if os.environ.get("KERNEL_NULL"):
        with TileContext(nc) as tc, ExitStack() as ctx:
            pool = ctx.enter_context(tc.tile_pool(name="p", bufs=1))
            z = pool.tile([128, 128], F32, tag="z")
            nc.sync.dma_start(z[:], hsT[0:128, 0:128])
            nc.sync.dma_start(qTh[0, :, 0:128], z[:])
            nc.sync.dma_start(kTs[:, 0:128], z[:])
            nc.sync.dma_start(ws[0:128, 0:64], z[:, 0:64])
        return nc

    KD = D // 128    # 56 contraction tiles over D
    KR = R // 128    # 12 contraction tiles over R
    with TileContext(nc) as tc, ExitStack() as ctx:
        cpool = ctx.enter_context(tc.tile_pool(name="const", bufs=1))
        hs_pool = ctx.enter_context(tc.tile_pool(name="hsres", bufs=1))
        ql_pool = ctx.enter_context(tc.tile_pool(name="qlt", bufs=2))
        work = ctx.enter_context(tc.tile_pool(name="work", bufs=2))
        small = ctx.enter_context(tc.tile_pool(name="small", bufs=2))
        evac = ctx.enter_context(tc.tile_pool(name="evac", bufs=3))
        ps_big = ctx.enter_context(tc.tile_pool(name="psbig", bufs=2, space="PSUM"))
        ps_t = ctx.enter_context(tc.tile_pool(name="pst", bufs=3, space="PSUM"))
        ps_k = ctx.enter_context(tc.tile_pool(name="psk", bufs=1, space="PSUM"))

        hm_sb = cpool.tile([128, 128], F32, tag="hm")
        nc.sync.dma_start(hm_sb[:], hm)
        id_sb = cpool.tile([128, 128], F32, tag="ident")
        nc.sync.dma_start(id_sb[:], ident)
        knw_sb = cpool.tile([128, HD], F32, tag="knw")
        nc.sync.dma_start(
            knw_sb[:],
            knw.rearrange("(o d) -> o d", o=1).to_broadcast([128, HD]))
        knb_sb = cpool.tile([128, HD], F32, tag="knb")
        nc.sync.dma_start(
            knb_sb[:],
            knb.rearrange("(o d) -> o d", o=1).to_broadcast([128, HD]))
        eps_sb = cpool.tile([128, 1], F32, tag="eps")
        nc.vector.memset(eps_sb[:], LN_EPS)

        # resident inputs, batched into few multi-level-AP DMAs
        hsT_sb = hs_pool.tile([128, KD * CH], F32, tag="hsT_sb")
        for dp in range(4):
            nc.sync.dma_start(
                hsT_sb[:, (KD // 4) * CH * dp:(KD // 4) * CH * (dp + 1)],
                hsT.rearrange("(kt p) c -> p kt c", p=128)[
                    :, (KD // 4) * dp:(KD // 4) * (dp + 1), :])
        hsT_v = hsT_sb[:].rearrange("p (kt c) -> p kt c", kt=KD)
        wqb_sb = cpool.tile([128, KR * HC * HD], F32, tag="wqb_sb")
        for dp in range(4):
            nc.sync.dma_start(
                wqb_sb[:, (KR // 4) * HC * HD * dp:(KR // 4) * HC * HD * (dp + 1)],
                wqb.rearrange("(kt p) c -> p kt c", p=128)[
                    :, (KR // 4) * dp:(KR // 4) * (dp + 1), :])
        wqb_v = wqb_sb[:].rearrange("p (kt c) -> p kt c", kt=KR)
        wk_sb = cpool.tile([128, KD * HD], F32, tag="wk_sb")
        nc.sync.dma_start(wk_sb[:],
                          wk.rearrange("(kt p) d -> p kt d", p=128))
        wk_v = wk_sb[:].rearrange("p (kt d) -> p kt d", kt=KD)
        wp_sb = cpool.tile([128, KD * H], F32, tag="wp_sb")
        nc.sync.dma_start(wp_sb[:],
                          wp.rearrange("(kt p) d -> p kt d", p=128))
        wp_v = wp_sb[:].rearrange("p (kt d) -> p kt d", kt=KD)

        # ---- k / w path over the 256-row chunk ----
        kT_fin = work.tile([128, CH], F32, tag="kT_fin")
        nc.vector.memset(kT_fin[:], 0.0)
        for rt in range(0 if os.environ.get("L1_SKIP_KW") else CH // 128):
            psk = ps_k.tile([128, HD], F32, tag="psk")
            psw = ps_k.tile([128, H], F32, tag="psw")
            for kt in range(KD):
                lhs = hsT_v[:, kt, 128 * rt:128 * (rt + 1)]
                nc.tensor.matmul(psk[:], lhs,
                                 wk_v[:, kt, :],
                                 start=(kt == 0), stop=(kt == KD - 1))
                nc.tensor.matmul(psw[:], lhs,
                                 wp_v[:, kt, :],
                                 start=(kt == 0), stop=(kt == KD - 1))
            k_rt = work.tile([128, HD], F32, tag="k_rt")
            nc.scalar.activation(k_rt[:], psk[:], AF.Copy)
            # layernorm over d
            mu = small.tile([128, 1], F32, tag="mu")
            nc.vector.tensor_reduce(mu[:], k_rt[:], axis=mybir.AxisListType.X,
                                    op=ALU.add)
            nc.vector.tensor_scalar_mul(mu[:], mu[:], 1.0 / HD)
            nc.vector.tensor_scalar_sub(k_rt[:], k_rt[:], mu[:])
            sq = work.tile([128, HD], F32, tag="sq")
            nc.vector.tensor_mul(sq[:], k_rt[:], k_rt[:])
            var = small.tile([128, 1], F32, tag="var")
            nc.vector.tensor_reduce(var[:], sq[:], axis=mybir.AxisListType.X,
                                    op=ALU.add)
            nc.vector.tensor_scalar_mul(var[:], var[:], 1.0 / HD)
            std = small.tile([128, 1], F32, tag="std")
            nc.scalar.activation(std[:], var[:], AF.Sqrt, bias=eps_sb[:])
            rstd = small.tile([128, 1], F32, tag="rstd")
            nc.vector.reciprocal(rstd[:], std[:])
            nc.vector.tensor_scalar_mul(k_rt[:], k_rt[:], rstd[:])
            nc.vector.tensor_mul(k_rt[:], k_rt[:], knw_sb[:])
            nc.vector.tensor_add(k_rt[:], k_rt[:], knb_sb[:])
            # rope (single "head")
            cs_t = small.tile([128, RO // 2], F32, tag="cs")
            nc.sync.dma_start(cs_t[:], csk[128 * rt:128 * (rt + 1), :])
            sn_t = small.tile([128, RO // 2], F32, tag="sn")
            nc.sync.dma_start(sn_t[:], snk[128 * rt:128 * (rt + 1), :])
            _rope_rows(nc, small, k_rt, 1, cs_t, sn_t)
            # transpose to [d, rows]
            pstr = ps_t.tile([128, 128], F32, tag="pt")
            nc.tensor.transpose(pstr[:], k_rt[:], id_sb[:])
            kTr = evac.tile([128, 128], F32, tag="kTr")
            nc.scalar.activation(kTr[:], pstr[:], AF.Copy)
            # hadamard (reuse the same PSUM tile)
            nc.tensor.matmul(pstr[:], hm_sb[:], kTr[:], start=True, stop=True)
            nc.scalar.activation(kT_fin[:, 128 * rt:128 * (rt + 1)], pstr[:],
                                 AF.Copy)
            # w output (scaled)
            w_sb = evac.tile([128, H], F32, tag="w_sb")
            nc.scalar.activation(w_sb[:], psw[:], AF.Copy, scale=SCALE_W)
            nc.sync.dma_start(ws[128 * rt:128 * (rt + 1), :], w_sb[:])
        nc.sync.dma_start(kTs[:, :], kT_fin[:])

        # ---- q path: all rows, HC heads ----
        # NOTE: k/w path is interleaved with q path by the Tile scheduler.
        nq_rt = 0 if os.environ.get("L1_SKIP_Q") else S // 128
        if nq_rt == 0:
            z = evac.tile([128, 128], F32, tag="qTo")
            nc.vector.memset(z[:], 0.0)
            for h in range(HC):
                for rt in range(S // 128):
                    nc.sync.dma_start(qTh[h, :, 128 * rt:128 * (rt + 1)], z[:])
        for rt in range(nq_rt):
            # wait: cos/sin tiles for these rows
            csq = small.tile([128, RO // 2], F32, tag="csq")
            nc.sync.dma_start(csq[:], cs[128 * rt:128 * (rt + 1), :])
            snq = small.tile([128, RO // 2], F32, tag="snq")
            nc.sync.dma_start(snq[:], sn[128 * rt:128 * (rt + 1), :])
            ql_sb = ql_pool.tile([128, KR * 128], F32, tag="ql_sb")
            nc.sync.dma_start(
                ql_sb[:],
                qlT.rearrange("(kt p) s -> p kt s", p=128)[
                    :, :, 128 * rt:128 * (rt + 1)])
            ql_v = ql_sb[:].rearrange("p (kt r) -> p kt r", kt=KR)
            q_rt = work.tile([128, HC * HD], F32, tag="q_rt")
            for ct in range(HC * HD // 512):
                psq = ps_big.tile([128, 512], F32, tag="psq")
                for kt in range(KR):
                    nc.tensor.matmul(psq[:], ql_v[:, kt, :],
                                     wqb_v[:, kt, 512 * ct:512 * (ct + 1)],
                                     start=(kt == 0), stop=(kt == KR - 1))
                nc.scalar.activation(q_rt[:, 512 * ct:512 * (ct + 1)], psq[:],
                                     AF.Copy)
            _rope_rows(nc, small, q_rt, HC, csq, snq)
            for g in range(HC // 4):
                pstq = ps_t.tile([128, 512], F32, tag="pt")
                for i in range(4):
                    h = 4 * g + i
                    nc.tensor.transpose(pstq[:, 128 * i:128 * (i + 1)],
                                        q_rt[:, 128 * h:128 * (h + 1)],
                                        id_sb[:])
                qTr = evac.tile([128, 512], F32, tag="qTr")
                nc.scalar.activation(qTr[:], pstq[:], AF.Copy)
                nc.tensor.matmul(pstq[:], hm_sb[:], qTr[:], start=True,
                                 stop=True)
                qTo = evac.tile([128, 512], F32, tag="qTo")
                nc.scalar.activation(qTo[:], pstq[:], AF.Copy)
                nc.sync.dma_start(
                    qTh[4 * g:4 * (g + 1), :,
                        128 * rt:128 * (rt + 1)].rearrange("h d r -> d h r"),
                    qTo[:].rearrange("p (h r) -> p h r", h=4))
    return nc


def build_l2():
    nc = bacc.Bacc("TRN2", target_bir_lowering=False, debug=False,
                   num_devices=NCORES)
    qp = nc.dram_tensor("qp", [2, 64, 128, 128], F32, kind="ExternalInput").ap()
    kT = nc.dram_tensor("kT", [HD, S], F32, kind="ExternalInput").ap()
    w22 = nc.dram_tensor("w22", [2, 128, 128], F32, kind="ExternalInput").ap()
    msk = nc.dram_tensor("msk", [sum(SLOT_TILES), 128, 512], F32,
                         kind="ExternalInput").ap()
    padidx = nc.dram_tensor("padidx", [2, 128, K], U32,
                            kind="ExternalInput").ap()
    rowl = nc.dram_tensor("rowl", [2, 128, 1], F32, kind="ExternalInput").ap()
    iotaf = nc.dram_tensor("iotaf", [128, K], F32, kind="ExternalInput").ap()
    sco = nc.dram_tensor("sco", [2, 128, S], F32, kind="ExternalOutput").ap()
    tki = nc.dram_tensor("tki", [2, 128, K], U32, kind="ExternalOutput").ap()

    if os.environ.get("KERNEL_NULL"):
        with TileContext(nc) as tc, ExitStack() as ctx:
            pool = ctx.enter_context(tc.tile_pool(name="p", bufs=1))
            z = pool.tile([128, 128], F32, tag="z")
            nc.sync.dma_start(z[:], kT[:, 0:128])
            for slot in range(2):
                nc.sync.dma_start(sco[slot, :, 0:128], z[:])
                nc.sync.dma_start(tki[slot, :, 0:128], z[:].bitcast(U32))
        return nc

    with TileContext(nc) as tc, ExitStack() as ctx:
        cpool = ctx.enter_context(tc.tile_pool(name="const", bufs=1))
        wpool = ctx.enter_context(tc.tile_pool(name="w22p", bufs=2))
        qpool = ctx.enter_context(tc.tile_pool(name="qp", bufs=4))
        rpool = ctx.enter_context(tc.tile_pool(name="relu", bufs=3))
        mpool = ctx.enter_context(tc.tile_pool(name="msk", bufs=2))
        spool = ctx.enter_context(tc.tile_pool(name="sc", bufs=2))
        ipool = ctx.enter_context(tc.tile_pool(name="idx", bufs=2))
        m8pool = ctx.enter_context(tc.tile_pool(name="m8", bufs=4))
        ps1 = ctx.enter_context(tc.tile_pool(name="ps1", bufs=3, space="PSUM"))
        ps2 = ctx.enter_context(tc.tile_pool(name="ps2", bufs=2, space="PSUM"))

        kT_sb = cpool.tile([128, S], F32, tag="kT")
        nc.sync.dma_start(kT_sb[:], kT)
        w2all = cpool.tile([128, 64 * 128], F32, tag="w2all")
        nc.vector.memset(w2all[:], 0.0)
        neginf = cpool.tile([128, 512], F32, tag="neginf")
        nc.vector.memset(neginf[:], float("-inf"))
        iotaf_sb = cpool.tile([128, K], F32, tag="iotaf")
        nc.sync.dma_start(iotaf_sb[:], iotaf)

        for slot in range(2):
            # whole slot's q operand resident: 8 DMAs instead of 384
            qp_sb = wpool.tile([128, 64 * 128], F32, tag="qp_sb")
            for dp in range(8):
                nc.sync.dma_start(
                    qp_sb[:, 1024 * dp:1024 * (dp + 1)],
                    qp[slot, 8 * dp:8 * (dp + 1)].rearrange("pr p m -> p pr m"))
            w22_sb = wpool.tile([128, 128], F32, tag="w22")
            nc.sync.dma_start(w22_sb[:], w22[slot])
            # scatter w into the block-diagonal MM2 weight layout:
            # w2all[(sp*64+h), 130*pair + sp] = w22[(sp*64+h), 2*pair + sp]
            nc.vector.tensor_copy(w2all[0:64, 0:8191:130],
                                  w22_sb[0:64, 0:128:2])
            nc.vector.tensor_copy(w2all[64:128, 1:8192:130],
                                  w22_sb[64:128, 1:128:2])
            scores_sb = spool.tile([128, S], F32, tag="scores")
            topk_wt = spool.tile([128, S], F32, tag="topkwt")
            ntt = SLOT_TILES[slot]
            for tt in range(ntt):
                p2 = ps2.tile([128, 512], F32, tag="p2")
                for pp in range(32):
                    p1 = ps1.tile([128, 1024], F32, tag="p1")
                    qa = qp_sb[:, 128 * (2 * pp):128 * (2 * pp + 1)]
                    qb = qp_sb[:, 128 * (2 * pp + 1):128 * (2 * pp + 2)]
                    kslice = kT_sb[:, 512 * tt:512 * (tt + 1)]
                    nc.tensor.matmul(p1[:, 0:512], qa,
                                     kslice, start=True,
                                     stop=True)
                    nc.tensor.matmul(p1[:, 512:1024], qb,
                                     kslice,
                                     start=True, stop=True)
                    rel = rpool.tile([128, 1024], F32, tag="rel")
                    nc.scalar.activation(rel[:], p1[:], AF.Relu)
                    nc.tensor.matmul(p2[:],
                                     w2all[:, 128 * (2 * pp):128 * (2 * pp + 1)],
                                     rel[:, 0:512],
                                     start=(pp == 0), stop=False)
                    nc.tensor.matmul(p2[:],
                                     w2all[:, 128 * (2 * pp + 1):128 * (2 * pp + 2)],
                                     rel[:, 512:1024],
                                     start=False, stop=(pp == 31))
                mi = mpool.tile([128, 512], F32, tag="mi")
                mask_idx = tt if slot == 0 else SLOT_TILES[0] + tt
                nc.sync.dma_start(mi[:], msk[mask_idx])
                nc.vector.tensor_add(scores_sb[:, 512 * tt:512 * (tt + 1)],
                                     p2[:], mi[:])
                nc.sync.dma_start(sco[slot, :, 512 * tt:512 * (tt + 1)],
                                  scores_sb[:, 512 * tt:512 * (tt + 1)])
                nc.vector.tensor_scalar_max(
                    topk_wt[:, 512 * tt:512 * (tt + 1)],
                    scores_sb[:, 512 * tt:512 * (tt + 1)], NEG_PAD)
            if slot == 0:
                for tt2 in range(ntt, S // 512):
                    nc.sync.dma_start(sco[slot, :, 512 * tt2:512 * (tt2 + 1)],
                                      neginf[:])
            # ---- top-K extraction ----
            idx_out = ipool.tile([128, K], U32, tag="idxout")
            wt_scan = topk_wt[:, 0:SLOT_SCAN[slot]]
            for it in range(K // 8):
                m8 = m8pool.tile([128, 8], F32, tag="m8")
                nc.vector.max(m8[:], wt_scan)
                nc.vector.max_index(idx_out[:, 8 * it:8 * (it + 1)], m8[:],
                                    wt_scan)
                if it < K // 8 - 1:
                    nc.vector.match_replace(wt_scan, m8[:], wt_scan, NEG_ZAP)
            # overwrite the causal-pad region with the reference's top_k
            # tie artifact pattern (precomputed on host per row)
            rl = wpool.tile([128, 1], F32, tag="rowl")
            nc.sync.dma_start(rl[:], rowl[slot])
            pm = spool.tile([128, K], mybir.dt.uint8, tag="pmask")
            nc.vector.tensor_scalar(pm[:], iotaf_sb[:], rl[:], None,
                                    op0=ALU.is_ge)
            pidx = ipool.tile([128, K], U32, tag="pidx")
            nc.sync.dma_start(pidx[:], padidx[slot])
            nc.vector.copy_predicated(idx_out[:], pm[:], pidx[:])
            nc.sync.dma_start(tki[slot], idx_out[:])
    return nc


def _hadamard_np():
    h = np.array([[1.0]])
    while h.shape[0] < HD:
        h = np.block([[h, h], [h, -h]])
    return (h * (HD ** -0.5)).astype(np.float32)


def _build_masks(core):
    m = np.empty((sum(SLOT_TILES), 128, 512), np.float32)
    col = np.arange(512)
    i = 0
    for slot, b in enumerate((core, core + 8)):
        row = 128 * b + np.arange(128)
        for tt in range(SLOT_TILES[slot]):
            valid = (512 * tt + col[None, :]) <= row[:, None]
            m[i] = np.where(valid, 0.0, -np.inf).astype(np.float32)
            i += 1
    return m


def _build_pads(core):
    """Reference jax.lax.top_k emits a deterministic tie pattern for the
    -inf (causal pad) region; reproduce it per row (host-precomputed)."""
    padidx = np.zeros((2, 128, K), np.uint32)
    rowl = np.zeros((2, 128, 1), np.float32)
    j = np.arange(K)
    for si, b in enumerate((core, core + 8)):
        for p in range(128):
            L = 128 * b + p + 1
            rowl[si, p, 0] = L
            if L < K:
                c8 = ((L + 7) // 8) * 8
                start = L if L < 8 else 0
                vals = np.where(j < c8, start + (j - L), (j - c8) % 8)
                padidx[si, p, :] = np.where(j >= L, vals, 0).astype(np.uint32)
    return padidx, rowl


_IOTAF = None


def _iotaf():
    global _IOTAF
    if _IOTAF is None:
        _IOTAF = np.tile(np.arange(K, dtype=np.float32), (128, 1))
    return _IOTAF


_CACHE = {}
LAST_RESULTS = []   # BassKernelResults of the most recent kernel() call


def _get(name, builder):
    if name not in _CACHE:
        nc = builder()
        nc.finalize()
        _CACHE[name] = nc
    return _CACHE[name]


def kernel(hidden_states, q_lora, cos, sin, wq_b, wk_w, k_norm_w, k_norm_b,
           weights_proj_w):
    hs = np.asarray(hidden_states, dtype=np.float32).reshape(S, D)
    ql = np.asarray(q_lora, dtype=np.float32).reshape(S, R)
    cosn = np.ascontiguousarray(np.asarray(cos, dtype=np.float32))
    sinn = np.ascontiguousarray(np.asarray(sin, dtype=np.float32))
    wqb = np.asarray(wq_b, dtype=np.float32)
    wk = np.ascontiguousarray(np.asarray(wk_w, dtype=np.float32))
    wp = np.ascontiguousarray(np.asarray(weights_proj_w, dtype=np.float32))
    knw = np.ascontiguousarray(np.asarray(k_norm_w, dtype=np.float32))
    knb = np.ascontiguousarray(np.asarray(k_norm_b, dtype=np.float32))

    hsT = np.ascontiguousarray(hs.T)
    qlT = np.ascontiguousarray(ql.T)
    hm = _hadamard_np()
    ident = np.eye(128, dtype=np.float32)

    in1 = []
    for c in range(NCORES):
        in1.append({
            "hsT": np.ascontiguousarray(hsT[:, c * CH:(c + 1) * CH]),
            "qlT": qlT,
            "wqb": np.ascontiguousarray(wqb[:, c * HC * HD:(c + 1) * HC * HD]),
            "wk": wk, "wp": wp, "cs": cosn, "sn": sinn,
            "csk": np.ascontiguousarray(cosn[c * CH:(c + 1) * CH]),
            "snk": np.ascontiguousarray(sinn[c * CH:(c + 1) * CH]),
            "knw": knw, "knb": knb, "hm": hm, "ident": ident,
        })
    nc1 = _get("l1", build_l1)
    LAST_RESULTS.clear()
    r1 = run_bass_kernel_spmd(nc1, in1, core_ids=list(range(NCORES)))
    LAST_RESULTS.append(r1)
    q_all = np.concatenate([r1.results[c]["qTh"] for c in range(NCORES)],
                           axis=0)                      # [64, 128, 2048]
    kTn = np.concatenate([r1.results[c]["kTs"] for c in range(NCORES)],
                         axis=1)                        # [128, 2048]
    w = np.concatenate([r1.results[c]["ws"] for c in range(NCORES)],
                       axis=0)                          # [2048, 64] scaled

    # qv[h, d, blk, pair, sp]
    qv = q_all.reshape(H, HD, 16, 64, 2)
    in2 = []
    for c in range(NCORES):
        qpk = np.empty((2, 64, 128, 128), np.float32)
        w22 = np.empty((2, 128, 128), np.float32)
        for si, b in enumerate((c, c + 8)):
            # [h, d, pair, sp] -> [pair, d, sp, h] -> [pair, d, sp*64+h]
            qpk[si] = np.ascontiguousarray(
                qv[:, :, b].transpose(2, 1, 3, 0)).reshape(64, 128, 128)
            w2T = w[128 * b:128 * (b + 1)].T            # [64, 128]
            w22[si] = np.concatenate([w2T, w2T], axis=0)
        pdx, rwl = _build_pads(c)
        in2.append({
            "qp": qpk, "kT": kTn, "w22": w22, "msk": _build_masks(c),
            "padidx": pdx, "rowl": rwl, "iotaf": _iotaf(),
        })
    nc2 = _get("l2", build_l2)
    r2 = run_bass_kernel_spmd(nc2, in2, core_ids=list(range(NCORES)))
    LAST_RESULTS.append(r2)

    scores = np.empty((S, S), np.float32)
    topk = np.empty((S, K), np.int32)
    for c in range(NCORES):
        for si, b in enumerate((c, c + 8)):
            scores[128 * b:128 * (b + 1)] = r2.results[c]["sco"][si]
            topk[128 * b:128 * (b + 1)] = (
                r2.results[c]["tki"][si].view(np.int32))
    return scores.reshape(1, S, S), topk.reshape(1, S, K)
